# revision 13
# baseline (speedup 1.0000x reference)
"""Trainium2 Bass kernel for nn_CPCModule (CPC loss_fn), SPMD over 8 NeuronCores.

Strategy (data-parallel over batch, b-reversed row order):
  - core j owns batch rows b in {15-2j, 14-2j}  (row index rho = 2j + p, b = 15 - rho)
  - conv1d via two K=5 matmuls on strided views; BN stats via ACT accum + AllReduce
  - q_k[b,t] = z[b,t] W_k z[b,t]^T + tr_b_k . z[b,t]  computed with float32r matmuls
    (positions on PSUM partitions) + DVE mul + ACT accumulate-reduce
  - negatives are a permuted view of q: g_neg_k[b,t] = q_k[(b-1)%16, perm[t]]
  - AllGather q -> every core permutes q by `perm` (indirect DMA row gather over a
    transposed copy) -> builds the torch-faithful flattened (11, l_len) -> (l_len, 11)
    softmax-group tensor per (b, k) pair -> log-sum-exp -> partial losses -> AllReduce.
"""

import os
import sys

sys.path.insert(0, "/opt/trn_rl_repo")

import numpy as np

import concourse.bass as bass
import concourse.tile as tile
from concourse import bacc, mybir
from concourse.bass import ds
from concourse.bass_utils import run_bass_kernel_spmd
from concourse.masks import make_identity

AF = mybir.ActivationFunctionType
OP = mybir.AluOpType
F32 = mybir.dt.float32
F32R = mybir.dt.float32r
I32 = mybir.dt.int32
AX = mybir.AxisListType.X

NCORES = 8
B, T, C, L = 16, 10240, 256, 2047
KS, NEG, S = 12, 10, 11
EPS = 1e-5
NB = 2                      # batch rows per core
VLEN = S * (L - 1)          # 11*2046, max flat length per (b,k) pair
G1N = 15 * 128              # groups covered by the full-size G tile


def _ap(t, offset, dims):
    return bass.AP(tensor=t, offset=offset, ap=[list(d) for d in dims])


def build_program():
    nc = bacc.Bacc("TRN2", target_bir_lowering=False, debug=False, num_devices=NCORES)

    x_sh = nc.dram_tensor("x_sh", [NB, T], F32, kind="ExternalInput")
    wc_in = nc.dram_tensor("wc", [10, C], F32, kind="ExternalInput")
    cb_in = nc.dram_tensor("cb", [C, 1], F32, kind="ExternalInput")
    gm_in = nc.dram_tensor("gm", [C, 1], F32, kind="ExternalInput")
    bt_in = nc.dram_tensor("bt", [C, 1], F32, kind="ExternalInput")
    trw_in = nc.dram_tensor("trw", [KS, C, C], F32, kind="ExternalInput")
    trb_in = nc.dram_tensor("trb", [KS, C], F32, kind="ExternalInput")
    perm_in = nc.dram_tensor("perm32", [2048, 1], I32, kind="ExternalInput")

    z_out = nc.dram_tensor("z_out", [NB, C, L], F32, kind="ExternalOutput")
    losses_out = nc.dram_tensor("losses", [KS, 1], F32, kind="ExternalOutput")

    divinv_np = np.array(
        [[1.0 / ((L - 2 * (kk + 1)) * B)] for kk in range(KS)], dtype=np.float32
    )
    divinv_d = nc.inline_tensor(divinv_np, name="divinv")

    with tile.TileContext(nc) as tc:
        with (
            tc.tile_pool(name="pp", bufs=1) as pp,
            tc.tile_pool(name="sp_", bufs=3) as sp,
            tc.tile_pool(name="smp", bufs=2) as smp,
            tc.tile_pool(name="ps", bufs=3, space="PSUM") as ps,
            tc.tile_pool(name="ps1", bufs=1, space="PSUM") as ps1,
            tc.tile_pool(name="dram", bufs=1, space="DRAM") as dram,
        ):
            # ---------------- phase 0: constants ----------------
            ident = pp.tile([128, 128], F32)
            make_identity(nc, ident[:])

            wcs = smp.tile([5, 2, C], F32)
            nc.sync.dma_start(wcs[:, 0, :], wc_in[0:5, :])
            nc.sync.dma_start(wcs[:, 1, :], wc_in[5:10, :])
            wcr = pp.tile([5, 2, C], F32R)
            nc.scalar.activation(wcr[:], wcs[:], AF.Copy)

            cbt = pp.tile([128, 2, 1], F32)
            gmt = pp.tile([128, 2, 1], F32)
            btt = pp.tile([128, 2, 1], F32)
            for t_, src in ((cbt, cb_in), (gmt, gm_in), (btt, bt_in)):
                nc.sync.dma_start(t_[:], _ap(src.ap().tensor, 0, [[1, 128], [128, 2], [1, 1]]))

            perm_sb = pp.tile([128, 16], I32)
            nc.sync.dma_start(perm_sb[:], _ap(perm_in.ap().tensor, 0, [[1, 128], [128, 16]]))

            dvt = pp.tile([KS, 1], F32)
            nc.sync.dma_start(dvt[:], divinv_d.ap())

            sums = pp.tile([128, 2, 8], F32)
            sumsq = pp.tile([128, 2, 8], F32)
            q_acc = pp.tile([128, NB, KS, 16], F32)
            acc_lse = pp.tile([128, KS], F32)
            acc_sv = pp.tile([128, KS], F32)
            nc.gpsimd.memset(acc_lse[:], 0.0)
            nc.gpsimd.memset(acc_sv[:], 0.0)
            onescol = pp.tile([128, 1], F32)
            nc.gpsimd.memset(onescol[:], 1.0)

            q_local = dram.tile([KS, NB, L], F32)

            with tc.tile_pool(name="pa", bufs=1) as pa:
                # -------- phase 0b: x load --------
                xt = pa.tile([5, NB, 2048], F32)
                for bl in range(NB):
                    nc.sync.dma_start(
                        xt[:, bl, :], x_sh[bl, :].rearrange("(u v) -> v u", v=5)
                    )
                xtr = pa.tile([5, NB, 2048], F32R)
                nc.scalar.activation(xtr[:], xt[:], AF.Copy)

                # -------- phase 1: conv -> relu(h) + stats --------
                hT = pa.tile([128, 2, NB, L], F32R)   # [c_local, blk, bl, l]
                for bl in range(NB):
                    for ch in range(4):
                        # chunk 3 overlaps by one position so every matmul is
                        # 512 wide (fp32r requires an even dst free size)
                        c0 = ch * 512 if ch < 3 else 1535
                        sk = 0 if ch < 3 else 1
                        pc = ps.tile([128, 2, 512], F32, tag="mm")
                        for blk in range(2):
                            lhs_a = wcr[:, 0, blk * 128:(blk + 1) * 128]
                            lhs_b = wcr[:, 1, blk * 128:(blk + 1) * 128]
                            nc.tensor.matmul(pc[:, blk, 0:512], lhs_a,
                                             xtr[:, bl, c0:c0 + 512], start=True, stop=False)
                            nc.tensor.matmul(pc[:, blk, 0:512], lhs_b,
                                             xtr[:, bl, c0 + 1:c0 + 513],
                                             start=False, stop=True)
                        for blk in range(2):
                            si = bl * 4 + ch
                            nc.scalar.activation(
                                hT[:, blk, bl, c0 + sk:c0 + 512], pc[:, blk, sk:512],
                                AF.Relu, bias=cbt[:, blk],
                                accum_out=sums[:, blk, si:si + 1],
                            )
                            sqs = sp.tile([128, 512], F32, tag="sqs")
                            nc.scalar.activation(
                                sqs[:, 0:512 - sk], hT[:, blk, bl, c0 + sk:c0 + 512],
                                AF.Square, accum_out=sumsq[:, blk, si:si + 1],
                            )

                # -------- phase 2: BN stats AllReduce + affine params --------
                stats = pp.tile([128, 2, 2], F32)
                nc.vector.reduce_sum(stats[:, :, 0:1], sums[:], axis=AX)
                nc.vector.reduce_sum(stats[:, :, 1:2], sumsq[:], axis=AX)
                stats_in = dram.tile([C, 2], F32)
                stats_out = dram.tile([C, 2], F32, addr_space="Shared")
                nc.sync.dma_start(
                    _ap(stats_in[:].tensor, 0, [[2, 128], [256, 2], [1, 2]]), stats[:]
                )
                nc.gpsimd.collective_compute(
                    "AllReduce", OP.add, replica_groups=[list(range(NCORES))],
                    ins=[stats_in[:].opt()], outs=[stats_out[:].opt()],
                )
                gstats = pp.tile([128, 2, 2], F32)
                nc.sync.dma_start(
                    gstats[:], _ap(stats_out[:].tensor, 0, [[2, 128], [256, 2], [1, 2]])
                )
                rcount = 1.0 / (B * L)
                mu = pp.tile([128, 2, 1], F32)
                ms = smp.tile([128, 2, 1], F32, tag="ms")
                var = smp.tile([128, 2, 1], F32, tag="var")
                rstd = smp.tile([128, 2, 1], F32, tag="rstd")
                avec = pp.tile([128, 2, 1], F32)
                bvec = pp.tile([128, 2, 1], F32)
                nc.vector.tensor_scalar_mul(mu[:], gstats[:, :, 0:1], rcount)
                nc.vector.tensor_scalar_mul(ms[:], gstats[:, :, 1:2], rcount)
                mu2 = smp.tile([128, 2, 1], F32, tag="mu2")
                nc.vector.tensor_tensor(mu2[:], mu[:], mu[:], op=OP.mult)
                nc.vector.tensor_tensor(var[:], ms[:], mu2[:], op=OP.subtract)
                sdev = smp.tile([128, 2, 1], F32, tag="sdev")
                nc.vector.tensor_scalar_add(var[:], var[:], EPS)
                nc.scalar.activation(sdev[:], var[:], AF.Sqrt)
                nc.vector.reciprocal(rstd[:], sdev[:])
                nc.vector.tensor_tensor(avec[:], gmt[:], rstd[:], op=OP.mult)
                mua = smp.tile([128, 2, 1], F32, tag="mua")
                nc.vector.tensor_tensor(mua[:], mu[:], avec[:], op=OP.mult)
                nc.vector.tensor_tensor(bvec[:], btt[:], mua[:], op=OP.subtract)

                # -------- phase 3: z (fp32r, in-place over h), z_out --------
                zT = hT[:]                            # in-place: same storage
                for blk in range(2):
                    nc.scalar.activation(zT[:, blk], hT[:, blk], AF.Identity,
                                         scale=avec[:, blk], bias=bvec[:, blk])
                    nc.sync.dma_start(
                        _ap(z_out.ap().tensor, blk * 128 * L,
                            [[L, 128], [C * L, NB], [1, L]]),
                        zT[:, blk].bitcast(F32),
                    )

                with tc.tile_pool(name="pb", bufs=1) as pb:
                    # WBr[:, kk*2+blk, :] = fp32r([W_kk[blk] | tr_b_kk[blk]])
                    WBr = pb.tile([128, 2 * KS, C + 2], F32R)
                    for kk in range(KS):
                        for blk in range(2):
                            wbs = sp.tile([128, C + 2], F32, tag="wbs")
                            nc.sync.dma_start(
                                wbs[:, 0:C], trw_in[kk, blk * 128:(blk + 1) * 128, :])
                            nc.sync.dma_start(
                                wbs[:, C:C + 1],
                                _ap(trb_in.ap().tensor, kk * C + blk * 128,
                                    [[1, 128], [1, 1]]),
                            )
                            nc.gpsimd.memset(wbs[:, C + 1:C + 2], 0.0)
                            nc.scalar.activation(WBr[:, kk * 2 + blk, :], wbs[:], AF.Copy)

                    # position-major z copies (PE transpose)
                    zPa = [pb.tile([128, 16, C + 2], F32, name=f"zPa{_bl}")
                           for _bl in range(NB)]
                    for bl in range(NB):
                        for c16 in range(16):
                            off = c16 * 128 if c16 < 15 else 1919
                            pt = ps.tile([128, 2, 512], F32, tag="mm")
                            for blk in range(2):
                                nc.tensor.transpose(
                                    pt[:, blk, 0:128],
                                    zT[:, blk, bl, off:off + 128].bitcast(F32), ident[:],
                                )
                                nc.scalar.activation(
                                    zPa[bl][:, c16, blk * 128:(blk + 1) * 128],
                                    pt[:, blk, 0:128], AF.Copy,
                                )
                            nc.gpsimd.memset(zPa[bl][:, c16, C:C + 2], 1.0)

                    # -------- phase 4: q quadratic forms --------
                    for bl in range(NB):
                        for c16 in range(16):
                            off = c16 * 128 if c16 < 15 else 1919
                            for kp in range(6):
                                Y2 = ps.tile([128, 2, 512], F32, tag="mm")
                                for t_ in range(2):
                                    kk = kp * 2 + t_
                                    nc.tensor.matmul(
                                        Y2[:, t_, 0:C + 2], zT[:, 0, bl, off:off + 128],
                                        WBr[:, kk * 2, :], start=True, stop=False)
                                    nc.tensor.matmul(
                                        Y2[:, t_, 0:C + 2], zT[:, 1, bl, off:off + 128],
                                        WBr[:, kk * 2 + 1, :], start=False, stop=True)
                                yz = sp.tile([128, 2, C + 2], F32, tag="yz")
                                nc.vector.tensor_tensor(
                                    yz[:], Y2[:, :, 0:C + 2],
                                    zPa[bl][:, c16].unsqueeze(1).to_broadcast(
                                        [128, 2, C + 2]),
                                    op=OP.mult)
                                for t_ in range(2):
                                    kk = kp * 2 + t_
                                    nc.scalar.activation(
                                        yz[:, t_, :], yz[:, t_, :], AF.Copy,
                                        accum_out=q_acc[:, bl, kk, c16:c16 + 1])

                    for bl in range(NB):
                        for kk in range(KS):
                            nc.sync.dma_start(
                                _ap(q_local[:].tensor, kk * NB * L + bl * L,
                                    [[1, 128], [128, 15]]),
                                q_acc[:, bl, kk, 0:15])
                        nc.sync.dma_start(
                            _ap(q_local[:].tensor, bl * L + 1920,
                                [[1, 127], [NB * L, KS]]),
                            q_acc[1:128, bl, :, 15:16])

            # ---------------- phase 5: AllGather q ------------------------------
            Q = dram.tile([NCORES, KS, NB, L], F32, addr_space="Shared")
            nc.gpsimd.collective_compute(
                "AllGather", OP.bypass, replica_groups=[list(range(NCORES))],
                ins=[q_local[:].opt()], outs=[Q[:].opt()],
            )

            # ---------------- phase 6: apply perm (replicated) ------------------
            # Q viewed as (192, L): row c = j*24 + kk*2 + p   (rho = 2j+p, b = 15-rho)
            qpe = dram.tile([KS, 26, L], F32)
            with tc.tile_pool(name="pc_", bufs=1) as pc_:
                TQ1 = pc_.tile([128, L], F32)
                TQ2 = pc_.tile([64, L], F32)
                nc.sync.dma_start(TQ1[:], _ap(Q[:].tensor, 0, [[L, 128], [1, L]]))
                nc.sync.dma_start(TQ2[:], _ap(Q[:].tensor, 128 * L, [[L, 64], [1, L]]))

                qTall = pc_.tile([128, 16, 192], F32)
                for sb_ in range(16):
                    ns = 128 if sb_ < 15 else 127
                    o = sb_ * 128
                    pt = ps.tile([128, 2, 512], F32, tag="mm")
                    nc.tensor.transpose(pt[0:ns, 0, 0:128], TQ1[:, o:o + ns], ident[:])
                    nc.scalar.activation(qTall[0:ns, sb_, 0:128],
                                         pt[0:ns, 0, 0:128], AF.Copy)
                    nc.tensor.transpose(pt[0:ns, 1, 0:64], TQ2[:, o:o + ns],
                                        ident[0:64, 0:64])
                    nc.scalar.activation(qTall[0:ns, sb_, 128:192],
                                         pt[0:ns, 1, 0:64], AF.Copy)

                qT_d = dram.tile([2048, 192], F32)
                nc.sync.dma_start(
                    _ap(qT_d[:].tensor, 0, [[192, 128], [128 * 192, 16], [1, 192]]),
                    qTall[:])

                qpTall = pc_.tile([128, 16, 192], F32)
                for sb_ in range(16):
                    nc.gpsimd.indirect_dma_start(
                        out=qpTall[:, sb_, :], out_offset=None, in_=qT_d[:],
                        in_offset=bass.IndirectOffsetOnAxis(
                            ap=perm_sb[:, sb_:sb_ + 1], axis=0),
                    )

                qp1 = pc_.tile([128, L], F32)   # cols c = 0..127  (j<=4, j=5 k<4)
                qp2 = pc_.tile([64, L], F32)    # cols c = 128..191
                for sb_ in range(16):
                    nt = 128 if sb_ < 15 else 127
                    o = sb_ * 128
                    pt = ps.tile([128, 2, 512], F32, tag="mm")
                    nc.tensor.transpose(pt[:, 0, 0:128], qpTall[:, sb_, 0:128], ident[:])
                    nc.scalar.activation(qp1[:, o:o + nt], pt[:, 0, 0:nt], AF.Copy)
                    nc.tensor.transpose(pt[0:64, 1, 0:128], qpTall[:, sb_, 128:192],
                                        ident[:])
                    nc.scalar.activation(qp2[:, o:o + nt], pt[0:64, 1, 0:nt], AF.Copy)

                # qp_ext[kk, m, t]: row m holds q_kk[b = 15-m mod 16, perm[t]]
                qpet = qpe[:].tensor
                for j in range(5):
                    nc.sync.dma_start(
                        _ap(qpet, 2 * j * L, [[26 * L, KS], [L, 2], [1, L]]),
                        qp1[j * 24:(j + 1) * 24, :])
                nc.sync.dma_start(
                    _ap(qpet, 10 * L, [[26 * L, 4], [L, 2], [1, L]]), qp1[120:128, :])
                nc.sync.dma_start(
                    _ap(qpet, 10 * L + 4 * 26 * L, [[26 * L, 8], [L, 2], [1, L]]),
                    qp2[0:16, :])
                for j in range(6, 8):
                    nc.sync.dma_start(
                        _ap(qpet, 2 * j * L, [[26 * L, KS], [L, 2], [1, L]]),
                        qp2[16 + (j - 6) * 24:16 + (j - 5) * 24, :])
                for j in range(5):
                    nc.sync.dma_start(
                        _ap(qpet, (16 + 2 * j) * L, [[26 * L, KS], [L, 2], [1, L]]),
                        qp1[j * 24:(j + 1) * 24, :])

            # ---------------- phase 7: V build + log-sum-exp --------------------
            rank = nc.sync.partition_id()
            V_all = dram.tile([NB, KS, VLEN], F32)

            def lse_tile(g_ap, P_, NG, kk):
                m1 = sp.tile([128, 15, 1], F32, tag="m1")
                nc.vector.reduce_max(m1[0:P_, 0:NG], g_ap, axis=AX)
                gs = sp.tile([128, 15, S], F32, tag="gs")
                nc.vector.tensor_tensor(
                    gs[0:P_, 0:NG], g_ap, m1[0:P_, 0:NG].to_broadcast([P_, NG, S]),
                    op=OP.subtract)
                ex = sp.tile([128, 15, S], F32, tag="ex")
                nc.scalar.activation(ex[0:P_, 0:NG], gs[0:P_, 0:NG], AF.Exp)
                ss = sp.tile([128, 15, 1], F32, tag="ss")
                nc.vector.reduce_sum(ss[0:P_, 0:NG], ex[0:P_, 0:NG], axis=AX)
                lns = sp.tile([128, 15, 1], F32, tag="lns")
                nc.scalar.activation(lns[0:P_, 0:NG], ss[0:P_, 0:NG], AF.Ln)
                lse = sp.tile([128, 15, 1], F32, tag="lse")
                nc.vector.tensor_tensor(lse[0:P_, 0:NG], lns[0:P_, 0:NG],
                                        m1[0:P_, 0:NG], op=OP.add)
                l1 = sp.tile([128, 1], F32, tag="l1")
                nc.vector.reduce_sum(l1[0:P_], lse[0:P_, 0:NG, 0], axis=AX)
                nc.vector.tensor_tensor(acc_lse[0:P_, kk:kk + 1],
                                        acc_lse[0:P_, kk:kk + 1], l1[0:P_], op=OP.add)
                svs = sp.tile([128, 15 * S], F32, tag="svs")
                sv1 = sp.tile([128, 1], F32, tag="sv1")
                nc.scalar.activation(svs[0:P_, 0:NG * S],
                                     g_ap.rearrange("p g e -> p (g e)"),
                                     AF.Copy, accum_out=sv1[0:P_])
                nc.vector.tensor_tensor(acc_sv[0:P_, kk:kk + 1],
                                        acc_sv[0:P_, kk:kk + 1], sv1[0:P_], op=OP.add)

            for par in range(NB):
                m0 = rank * 2 + (par + 1)
                for kk in range(KS):
                    kh = kk + 1
                    ll = L - kh
                    n2 = ll - G1N
                    nc.sync.dma_start(V_all[par, kk, 0:ll], q_local[kk, par, kh:L])
                    nc.sync.dma_start(V_all[par, kk, ll:S * ll],
                                      qpe[kk, ds(m0, 10), kh:kh + ll])
                    g1 = sp.tile([128, 15, S], F32, tag="g1")
                    nc.sync.dma_start(
                        g1[:],
                        V_all[par, kk, 0:G1N * S].rearrange(
                            "(g2 g1 e) -> g1 g2 e", g1=128, e=S))
                    g2 = sp.tile([128, 1, S], F32, tag="g2")
                    nc.sync.dma_start(
                        g2[0:n2, 0, :],
                        V_all[par, kk, G1N * S:(G1N + n2) * S].rearrange(
                            "(g e) -> g e", e=S))
                    lse_tile(g1[:, :, :], 128, 15, kk)
                    lse_tile(g2[0:n2, :, :], n2, 1, kk)

            # ---------------- phase 8: partial losses + AllReduce ---------------
            pl = ps1.tile([KS, 2], F32)
            nc.tensor.matmul(pl[:, 0:1], acc_lse[:], onescol[:], start=True, stop=True)
            nc.tensor.matmul(pl[:, 1:2], acc_sv[:], onescol[:], start=True, stop=True)
            lsb = smp.tile([KS, 2], F32, tag="lsb")
            nc.scalar.activation(lsb[:], pl[:], AF.Copy)
            t1 = smp.tile([KS, 1], F32, tag="t1")
            nc.vector.tensor_scalar_mul(t1[:], lsb[:, 0:1], float(S))
            t2 = smp.tile([KS, 1], F32, tag="t2")
            nc.vector.tensor_tensor(t2[:], t1[:], lsb[:, 1:2], op=OP.subtract)
            plosses = smp.tile([KS, 1], F32, tag="plo")
            nc.vector.tensor_tensor(plosses[:], t2[:], dvt[:], op=OP.mult)
            lin = dram.tile([KS, 1], F32)
            lout = dram.tile([KS, 1], F32, addr_space="Shared")
            nc.sync.dma_start(lin[:], plosses[:])
            nc.gpsimd.collective_compute(
                "AllReduce", OP.add, replica_groups=[list(range(NCORES))],
                ins=[lin[:].opt()], outs=[lout[:].opt()],
            )
            nc.sync.dma_start(losses_out.ap(), lout[:])

    nc.compile()
    return nc


_NC = None


def _get_nc():
    global _NC
    if _NC is None:
        _NC = build_program()
    return _NC


def kernel(x, conv_w, conv_b, bn_gamma, bn_beta, tr_w, tr_b, perm, _trace=False):
    x = np.asarray(x, np.float32)
    conv_w = np.asarray(conv_w, np.float32)
    conv_b = np.asarray(conv_b, np.float32)
    bn_gamma = np.asarray(bn_gamma, np.float32)
    bn_beta = np.asarray(bn_beta, np.float32)
    tr_w = np.ascontiguousarray(np.asarray(tr_w, np.float32))
    tr_b = np.ascontiguousarray(np.asarray(tr_b, np.float32))
    perm = np.asarray(perm)

    wc = np.ascontiguousarray(conv_w[:, 0, :].T)          # (10, 256)
    cb = np.ascontiguousarray(conv_b.reshape(C, 1))
    gm = np.ascontiguousarray(bn_gamma.reshape(C, 1))
    bt = np.ascontiguousarray(bn_beta.reshape(C, 1))
    perm32 = np.zeros((2048, 1), np.int32)
    perm32[:L, 0] = perm.astype(np.int32)

    in_maps = []
    for j in range(NCORES):
        xs = np.ascontiguousarray(
            np.stack([x[15 - 2 * j, 0, :], x[14 - 2 * j, 0, :]]))
        in_maps.append({
            "x_sh": xs, "wc": wc, "cb": cb, "gm": gm, "bt": bt,
            "trw": tr_w, "trb": tr_b, "perm32": perm32,
        })

    nc = _get_nc()
    res = run_bass_kernel_spmd(nc, in_maps, core_ids=list(range(NCORES)),
                               trace=_trace)

    z_full = np.empty((B, C, L), np.float32)
    for j in range(NCORES):
        zo = res.results[j]["z_out"]
        z_full[15 - 2 * j] = zo[0]
        z_full[14 - 2 * j] = zo[1]
    losses = np.ascontiguousarray(res.results[0]["losses"][:, 0])
    if _trace:
        kernel.last_exec_time_ns = res.exec_time_ns
    return z_full, losses


# revision 19
# speedup vs baseline: 1.0975x; 1.0975x over previous
"""Trainium2 Bass kernel for nn_CPCModule (CPC loss_fn), SPMD over 8 NeuronCores.

Strategy (data-parallel over batch, b-reversed row order):
  - core j owns batch rows b in {15-2j, 14-2j}  (row index rho = 2j + p, b = 15 - rho)
  - conv1d via two K=5 matmuls on strided views; BN stats via ACT accum + AllReduce
  - q_k[b,t] = z[b,t] W_k z[b,t]^T + tr_b_k . z[b,t]  computed with float32r matmuls
    (positions on PSUM partitions) + DVE mul + ACT accumulate-reduce
  - negatives are a permuted view of q: g_neg_k[b,t] = q_k[(b-1)%16, perm[t]]
  - AllGather q -> every core permutes q by `perm` (indirect DMA row gather over a
    transposed copy) -> builds the torch-faithful flattened (11, l_len) -> (l_len, 11)
    softmax-group tensor per (b, k) pair -> log-sum-exp -> partial losses -> AllReduce.
"""

import os
import sys

sys.path.insert(0, "/opt/trn_rl_repo")

import numpy as np

import concourse.bass as bass
import concourse.tile as tile
from concourse import bacc, mybir
from concourse.bass import ds
from concourse.bass_utils import run_bass_kernel_spmd
from concourse.masks import make_identity


AF = mybir.ActivationFunctionType
OP = mybir.AluOpType
F32 = mybir.dt.float32
F32R = mybir.dt.float32r
I32 = mybir.dt.int32
AX = mybir.AxisListType.X

NCORES = 8
B, T, C, L = 16, 10240, 256, 2047
KS, NEG, S = 12, 10, 11
EPS = 1e-5
NB = 2                      # batch rows per core
VLEN = S * (L - 1)          # 11*2046, max flat length per (b,k) pair
G1N = 15 * 128              # groups covered by the full-size G tile


def _ap(t, offset, dims):
    return bass.AP(tensor=t, offset=offset, ap=[list(d) for d in dims])


def build_program(sim=False):
    nc = bacc.Bacc("TRN2", target_bir_lowering=False, debug=False, num_devices=NCORES)

    x_sh = nc.dram_tensor("x_sh", [NB, T], F32, kind="ExternalInput")
    wc_in = nc.dram_tensor("wc", [10, C], F32, kind="ExternalInput")
    cb_in = nc.dram_tensor("cb", [C, 1], F32, kind="ExternalInput")
    gm_in = nc.dram_tensor("gm", [C, 1], F32, kind="ExternalInput")
    bt_in = nc.dram_tensor("bt", [C, 1], F32, kind="ExternalInput")
    trw_in = nc.dram_tensor("trw", [KS, C, C], F32, kind="ExternalInput")
    trb_in = nc.dram_tensor("trb", [KS, C], F32, kind="ExternalInput")
    perm_in = nc.dram_tensor("perm32", [2048, 1], I32, kind="ExternalInput")

    z_out = nc.dram_tensor("z_out", [NB, C, L], F32, kind="ExternalOutput")
    losses_out = nc.dram_tensor("losses", [KS, 1], F32, kind="ExternalOutput")

    divinv_np = np.array(
        [[1.0 / ((L - 2 * (kk + 1)) * B)] for kk in range(KS)], dtype=np.float32
    )
    divinv_d = nc.inline_tensor(divinv_np, name="divinv")

    with tile.TileContext(nc) as tc:
        with (
            tc.tile_pool(name="pp", bufs=1) as pp,
            tc.tile_pool(name="sp_", bufs=3) as sp,
            tc.tile_pool(name="smp", bufs=2) as smp,
            tc.tile_pool(name="ps", bufs=3, space="PSUM") as ps,
            tc.tile_pool(name="ps1", bufs=1, space="PSUM") as ps1,
            tc.tile_pool(name="dram", bufs=1, space="DRAM") as dram,
        ):
            # ---------------- phase 0: constants ----------------
            ident = pp.tile([128, 128], F32)
            make_identity(nc, ident[:])

            wcs = smp.tile([5, 2, C], F32)
            nc.sync.dma_start(wcs[:, 0, :], wc_in[0:5, :])
            nc.sync.dma_start(wcs[:, 1, :], wc_in[5:10, :])
            wcr = pp.tile([5, 2, C], F32R)
            nc.scalar.activation(wcr[:], wcs[:], AF.Copy)

            cbt = pp.tile([128, 2, 1], F32)
            gmt = pp.tile([128, 2, 1], F32)
            btt = pp.tile([128, 2, 1], F32)
            for t_, src in ((cbt, cb_in), (gmt, gm_in), (btt, bt_in)):
                nc.sync.dma_start(t_[:], _ap(src.ap().tensor, 0, [[1, 128], [128, 2], [1, 1]]))

            perm_sb = pp.tile([128, 16], I32)
            nc.sync.dma_start(perm_sb[:], _ap(perm_in.ap().tensor, 0, [[1, 128], [128, 16]]))

            dvt = pp.tile([KS, 1], F32)
            nc.sync.dma_start(dvt[:], divinv_d.ap())

            sums = pp.tile([128, 2, 8], F32)
            sumsq = pp.tile([128, 2, 8], F32)
            q_acc = pp.tile([128, NB, KS, 16], F32)
            acc_lse = pp.tile([128, KS], F32)
            acc_sv = pp.tile([128, KS], F32)
            nc.gpsimd.memset(acc_lse[:], 0.0)
            nc.gpsimd.memset(acc_sv[:], 0.0)
            onescol = pp.tile([128, 1], F32)
            nc.gpsimd.memset(onescol[:], 1.0)

            q_local = dram.tile([KS, NB, L], F32)

            with tc.tile_pool(name="pa", bufs=1) as pa:
                # -------- phase 0b: x load --------
                xt = pa.tile([5, NB, 2048], F32)
                for bl in range(NB):
                    nc.sync.dma_start(
                        xt[:, bl, :], x_sh[bl, :].rearrange("(u v) -> v u", v=5)
                    )
                xtr = pa.tile([5, NB, 2048], F32R)
                nc.scalar.activation(xtr[:], xt[:], AF.Copy)

                # -------- phase 1: conv -> relu(h) + stats --------
                hT = pa.tile([128, 2, NB, L], F32R)   # [c_local, blk, bl, l]
                for bl in range(NB):
                    for ch in range(4):
                        # chunk 3 overlaps by one position so every matmul is
                        # 512 wide (fp32r requires an even dst free size)
                        c0 = ch * 512 if ch < 3 else 1535
                        sk = 0 if ch < 3 else 1
                        pc = ps.tile([128, 2, 512], F32, tag="mm")
                        for blk in range(2):
                            lhs_a = wcr[:, 0, blk * 128:(blk + 1) * 128]
                            lhs_b = wcr[:, 1, blk * 128:(blk + 1) * 128]
                            nc.tensor.matmul(pc[:, blk, 0:512], lhs_a,
                                             xtr[:, bl, c0:c0 + 512], start=True, stop=False)
                            nc.tensor.matmul(pc[:, blk, 0:512], lhs_b,
                                             xtr[:, bl, c0 + 1:c0 + 513],
                                             start=False, stop=True)
                        for blk in range(2):
                            si = bl * 4 + ch
                            nc.scalar.activation(
                                hT[:, blk, bl, c0 + sk:c0 + 512], pc[:, blk, sk:512],
                                AF.Relu, bias=cbt[:, blk],
                                accum_out=sums[:, blk, si:si + 1],
                            )
                            sqs = sp.tile([128, 512], F32, tag="sqs")
                            hsl = hT[:, blk, bl, c0 + sk:c0 + 512].bitcast(F32)
                            nc.vector.tensor_tensor(sqs[:, 0:512 - sk], hsl, hsl,
                                                    op=OP.mult)
                            nc.vector.reduce_sum(sumsq[:, blk, si:si + 1],
                                                 sqs[:, 0:512 - sk], axis=AX)

                # -------- phase 2: BN stats AllReduce + affine params --------
                stats = pp.tile([128, 2, 2], F32)
                nc.vector.reduce_sum(stats[:, :, 0:1], sums[:], axis=AX)
                nc.vector.reduce_sum(stats[:, :, 1:2], sumsq[:], axis=AX)
                stats_in = dram.tile([C, 2], F32)
                stats_out = dram.tile([C, 2], F32, addr_space="Shared")
                nc.sync.dma_start(
                    _ap(stats_in[:].tensor, 0, [[2, 128], [256, 2], [1, 2]]), stats[:]
                )
                if sim:
                    nc.sync.dma_start(stats_out[:], stats_in[:])
                else:
                    nc.gpsimd.collective_compute(
                        "AllReduce", OP.add, replica_groups=[list(range(NCORES))],
                        ins=[stats_in[:].opt()], outs=[stats_out[:].opt()],
                    )
                gstats = pp.tile([128, 2, 2], F32)
                nc.sync.dma_start(
                    gstats[:], _ap(stats_out[:].tensor, 0, [[2, 128], [256, 2], [1, 2]])
                )
                rcount = 1.0 / (B * L)
                mu = pp.tile([128, 2, 1], F32)
                ms = smp.tile([128, 2, 1], F32, tag="ms")
                var = smp.tile([128, 2, 1], F32, tag="var")
                rstd = smp.tile([128, 2, 1], F32, tag="rstd")
                avec = pp.tile([128, 2, 1], F32)
                bvec = pp.tile([128, 2, 1], F32)
                nc.vector.tensor_scalar_mul(mu[:], gstats[:, :, 0:1], rcount)
                nc.vector.tensor_scalar_mul(ms[:], gstats[:, :, 1:2], rcount)
                mu2 = smp.tile([128, 2, 1], F32, tag="mu2")
                nc.vector.tensor_tensor(mu2[:], mu[:], mu[:], op=OP.mult)
                nc.vector.tensor_tensor(var[:], ms[:], mu2[:], op=OP.subtract)
                lnv = smp.tile([128, 2, 1], F32, tag="lnv")
                nc.vector.tensor_scalar_add(var[:], var[:], EPS)
                nc.scalar.activation(lnv[:], var[:], AF.Ln)
                nc.scalar.activation(rstd[:], lnv[:], AF.Exp, scale=-0.5)
                nc.vector.tensor_tensor(avec[:], gmt[:], rstd[:], op=OP.mult)
                mua = smp.tile([128, 2, 1], F32, tag="mua")
                nc.vector.tensor_tensor(mua[:], mu[:], avec[:], op=OP.mult)
                nc.vector.tensor_tensor(bvec[:], btt[:], mua[:], op=OP.subtract)

                # -------- phase 3: z (fp32r, in-place over h), z_out --------
                zT = hT[:]                            # in-place: same storage
                for blk in range(2):
                    nc.scalar.activation(zT[:, blk], hT[:, blk], AF.Identity,
                                         scale=avec[:, blk], bias=bvec[:, blk])
                    nc.sync.dma_start(
                        _ap(z_out.ap().tensor, blk * 128 * L,
                            [[L, 128], [C * L, NB], [1, L]]),
                        zT[:, blk].bitcast(F32),
                    )

                with tc.tile_pool(name="pb", bufs=1) as pb:
                    # WBr[:, kk*2+blk, :] = fp32r([W_kk[blk] | tr_b_kk[blk]])
                    WBr = pb.tile([128, 2 * KS, C + 2], F32R)
                    for kk in range(KS):
                        for blk in range(2):
                            wbs = sp.tile([128, C + 2], F32, tag="wbs")
                            nc.sync.dma_start(
                                wbs[:, 0:C], trw_in[kk, blk * 128:(blk + 1) * 128, :])
                            nc.sync.dma_start(
                                wbs[:, C:C + 1],
                                _ap(trb_in.ap().tensor, kk * C + blk * 128,
                                    [[1, 128], [1, 1]]),
                            )
                            nc.gpsimd.memset(wbs[:, C + 1:C + 2], 0.0)
                            nc.scalar.activation(WBr[:, kk * 2 + blk, :], wbs[:], AF.Copy)

                    # position-major z copies (PE transpose)
                    zPa = [pb.tile([128, 16, C + 2], F32, name=f"zPa{_bl}")
                           for _bl in range(NB)]
                    for bl in range(NB):
                        for c16 in range(16):
                            off = c16 * 128 if c16 < 15 else 1919
                            pt = ps.tile([128, 2, 512], F32, tag="mm")
                            for blk in range(2):
                                nc.tensor.transpose(
                                    pt[:, blk, 0:128],
                                    zT[:, blk, bl, off:off + 128].bitcast(F32), ident[:],
                                )
                                nc.vector.tensor_copy(
                                    zPa[bl][:, c16, blk * 128:(blk + 1) * 128],
                                    pt[:, blk, 0:128],
                                )
                            nc.gpsimd.memset(zPa[bl][:, c16, C:C + 2], 1.0)

                    # -------- phase 4: q quadratic forms --------
                    for bl in range(NB):
                        for c16 in range(16):
                            off = c16 * 128 if c16 < 15 else 1919
                            yza = sp.tile([128, KS, C + 2], F32, tag="yza", bufs=2)
                            for kp in range(6):
                                Y2 = ps.tile([128, 2, 512], F32, tag="mm")
                                for t_ in range(2):
                                    kk = kp * 2 + t_
                                    nc.tensor.matmul(
                                        Y2[:, t_, 0:C + 2], zT[:, 0, bl, off:off + 128],
                                        WBr[:, kk * 2, :], start=True, stop=False)
                                    nc.tensor.matmul(
                                        Y2[:, t_, 0:C + 2], zT[:, 1, bl, off:off + 128],
                                        WBr[:, kk * 2 + 1, :], start=False, stop=True)
                                nc.vector.tensor_tensor(
                                    yza[:, kp * 2:kp * 2 + 2, :], Y2[:, :, 0:C + 2],
                                    zPa[bl][:, c16].unsqueeze(1).to_broadcast(
                                        [128, 2, C + 2]),
                                    op=OP.mult)
                            if (bl * 16 + c16) % 3 == 0:
                                nc.vector.reduce_sum(
                                    q_acc[:, bl, :, c16:c16 + 1], yza[:], axis=AX)
                            else:
                                for kk in range(KS):
                                    nc.scalar.activation(
                                        yza[:, kk, :], yza[:, kk, :], AF.Copy,
                                        accum_out=q_acc[:, bl, kk, c16:c16 + 1])

                    for bl in range(NB):
                        for kk in range(KS):
                            nc.sync.dma_start(
                                _ap(q_local[:].tensor, kk * NB * L + bl * L,
                                    [[1, 128], [128, 15]]),
                                q_acc[:, bl, kk, 0:15])
                        nc.sync.dma_start(
                            _ap(q_local[:].tensor, bl * L + 1920,
                                [[1, 127], [NB * L, KS]]),
                            q_acc[1:128, bl, :, 15:16])

            # ---------------- phase 5: AllGather q ------------------------------
            Q = dram.tile([NCORES, KS, NB, L], F32, addr_space="Shared")
            if sim:
                nc.sync.dma_start(Q[0], q_local[:])
            else:
                nc.gpsimd.collective_compute(
                    "AllGather", OP.bypass, replica_groups=[list(range(NCORES))],
                    ins=[q_local[:].opt()], outs=[Q[:].opt()],
                )

            # ---------------- phase 6: apply perm (replicated) ------------------
            # Q viewed as (192, L): row c = j*24 + kk*2 + p   (rho = 2j+p, b = 15-rho)
            qpe = dram.tile([KS, 26, L], F32)
            with tc.tile_pool(name="pc_", bufs=1) as pc_:
                TQ1 = pc_.tile([128, L], F32)
                TQ2 = pc_.tile([64, L], F32)
                nc.sync.dma_start(TQ1[:], _ap(Q[:].tensor, 0, [[L, 128], [1, L]]))
                nc.sync.dma_start(TQ2[:], _ap(Q[:].tensor, 128 * L, [[L, 64], [1, L]]))

                qTall = pc_.tile([128, 16, 192], F32)
                for sb_ in range(16):
                    ns = 128 if sb_ < 15 else 127
                    o = sb_ * 128
                    pt = ps.tile([128, 2, 512], F32, tag="mm")
                    nc.tensor.transpose(pt[0:ns, 0, 0:128], TQ1[:, o:o + ns], ident[:])
                    nc.scalar.activation(qTall[0:ns, sb_, 0:128],
                                         pt[0:ns, 0, 0:128], AF.Copy)
                    nc.tensor.transpose(pt[0:ns, 1, 0:64], TQ2[:, o:o + ns],
                                        ident[0:64, 0:64])
                    nc.scalar.activation(qTall[0:ns, sb_, 128:192],
                                         pt[0:ns, 1, 0:64], AF.Copy)

                qT_d = dram.tile([2048, 192], F32)
                nc.sync.dma_start(
                    _ap(qT_d[:].tensor, 0, [[192, 128], [128 * 192, 16], [1, 192]]),
                    qTall[:])

                qpTall = pc_.tile([128, 16, 192], F32)
                for sb_ in range(16):
                    nc.gpsimd.indirect_dma_start(
                        out=qpTall[:, sb_, :], out_offset=None, in_=qT_d[:],
                        in_offset=bass.IndirectOffsetOnAxis(
                            ap=perm_sb[:, sb_:sb_ + 1], axis=0),
                    )

                qp1 = pc_.tile([128, L], F32)   # cols c = 0..127  (j<=4, j=5 k<4)
                qp2 = pc_.tile([64, L], F32)    # cols c = 128..191
                for sb_ in range(16):
                    nt = 128 if sb_ < 15 else 127
                    o = sb_ * 128
                    pt = ps.tile([128, 2, 512], F32, tag="mm")
                    nc.tensor.transpose(pt[:, 0, 0:128], qpTall[:, sb_, 0:128], ident[:])
                    nc.scalar.activation(qp1[:, o:o + nt], pt[:, 0, 0:nt], AF.Copy)
                    nc.tensor.transpose(pt[0:64, 1, 0:128], qpTall[:, sb_, 128:192],
                                        ident[:])
                    nc.scalar.activation(qp2[:, o:o + nt], pt[0:64, 1, 0:nt], AF.Copy)

                # qp_ext[kk, m, t]: row m holds q_kk[b = 15-m mod 16, perm[t]]
                qpet = qpe[:].tensor
                for j in range(5):
                    nc.sync.dma_start(
                        _ap(qpet, 2 * j * L, [[26 * L, KS], [L, 2], [1, L]]),
                        qp1[j * 24:(j + 1) * 24, :])
                nc.sync.dma_start(
                    _ap(qpet, 10 * L, [[26 * L, 4], [L, 2], [1, L]]), qp1[120:128, :])
                nc.sync.dma_start(
                    _ap(qpet, 10 * L + 4 * 26 * L, [[26 * L, 8], [L, 2], [1, L]]),
                    qp2[0:16, :])
                for j in range(6, 8):
                    nc.sync.dma_start(
                        _ap(qpet, 2 * j * L, [[26 * L, KS], [L, 2], [1, L]]),
                        qp2[16 + (j - 6) * 24:16 + (j - 5) * 24, :])
                for j in range(5):
                    nc.sync.dma_start(
                        _ap(qpet, (16 + 2 * j) * L, [[26 * L, KS], [L, 2], [1, L]]),
                        qp1[j * 24:(j + 1) * 24, :])

            # ---------------- phase 7: V build + log-sum-exp --------------------
            rank = nc.sync.partition_id()
            V_tiles = [[dram.tile([VLEN], F32, name=f"Vp{_p}_{_k}")
                        for _k in range(KS)] for _p in range(NB)]

            def lse_tile(g_ap, P_, NG, kk):
                m1 = sp.tile([128, 15, 1], F32, tag="m1")
                nc.vector.reduce_max(m1[0:P_, 0:NG], g_ap, axis=AX)
                gs = sp.tile([128, 15, S], F32, tag="gs")
                nc.gpsimd.tensor_tensor(
                    gs[0:P_, 0:NG], g_ap, m1[0:P_, 0:NG].to_broadcast([P_, NG, S]),
                    op=OP.subtract)
                ex = sp.tile([128, 15, S], F32, tag="ex")
                nc.scalar.activation(ex[0:P_, 0:NG], gs[0:P_, 0:NG], AF.Exp)
                ss = sp.tile([128, 15, 1], F32, tag="ss")
                nc.vector.reduce_sum(ss[0:P_, 0:NG], ex[0:P_, 0:NG], axis=AX)
                lns = sp.tile([128, 15, 1], F32, tag="lns")
                nc.scalar.activation(lns[0:P_, 0:NG], ss[0:P_, 0:NG], AF.Ln)
                lse = sp.tile([128, 15, 1], F32, tag="lse")
                nc.gpsimd.tensor_tensor(lse[0:P_, 0:NG], lns[0:P_, 0:NG],
                                        m1[0:P_, 0:NG], op=OP.add)
                l1 = sp.tile([128, 1], F32, tag="l1")
                nc.vector.reduce_sum(l1[0:P_], lse[0:P_, 0:NG, 0], axis=AX)
                nc.gpsimd.tensor_tensor(acc_lse[0:P_, kk:kk + 1],
                                        acc_lse[0:P_, kk:kk + 1], l1[0:P_], op=OP.add)
                sv2 = sp.tile([128, 15, 1], F32, tag="sv2")
                nc.vector.reduce_sum(sv2[0:P_, 0:NG], g_ap, axis=AX)
                sv1 = sp.tile([128, 1], F32, tag="sv1")
                nc.vector.reduce_sum(sv1[0:P_], sv2[0:P_, 0:NG, 0], axis=AX)
                nc.gpsimd.tensor_tensor(acc_sv[0:P_, kk:kk + 1],
                                        acc_sv[0:P_, kk:kk + 1], sv1[0:P_], op=OP.add)

            for par in range(NB):
                m0 = rank * 2 + (par + 1)
                for kk in range(KS):
                    kh = kk + 1
                    ll = L - kh
                    n2 = ll - G1N
                    Vb = V_tiles[par][kk]
                    nc.sync.dma_start(Vb[0:ll], q_local[kk, par, kh:L])
                    nc.sync.dma_start(Vb[ll:S * ll],
                                      qpe[kk, ds(m0, 10), kh:kh + ll])
                    g1 = sp.tile([128, 15, S], F32, tag="g1")
                    nc.scalar.dma_start(
                        g1[:],
                        Vb[0:G1N * S].rearrange(
                            "(g1 g2 e) -> g1 g2 e", g2=15, e=S))
                    g2 = sp.tile([128, 1, S], F32, tag="g2")
                    nc.scalar.dma_start(
                        g2[0:n2, 0, :],
                        Vb[G1N * S:(G1N + n2) * S].rearrange(
                            "(g e) -> g e", e=S))
                    lse_tile(g1[:, :, :], 128, 15, kk)
                    lse_tile(g2[0:n2, :, :], n2, 1, kk)

            # ---------------- phase 8: partial losses + AllReduce ---------------
            pl = ps1.tile([KS, 2], F32)
            nc.tensor.matmul(pl[:, 0:1], acc_lse[:], onescol[:], start=True, stop=True)
            nc.tensor.matmul(pl[:, 1:2], acc_sv[:], onescol[:], start=True, stop=True)
            lsb = smp.tile([KS, 2], F32, tag="lsb")
            nc.scalar.activation(lsb[:], pl[:], AF.Copy)
            t1 = smp.tile([KS, 1], F32, tag="t1")
            nc.vector.tensor_scalar_mul(t1[:], lsb[:, 0:1], float(S))
            t2 = smp.tile([KS, 1], F32, tag="t2")
            nc.vector.tensor_tensor(t2[:], t1[:], lsb[:, 1:2], op=OP.subtract)
            plosses = smp.tile([KS, 1], F32, tag="plo")
            nc.vector.tensor_tensor(plosses[:], t2[:], dvt[:], op=OP.mult)
            lin = dram.tile([KS, 1], F32)
            lout = dram.tile([KS, 1], F32, addr_space="Shared")
            nc.sync.dma_start(lin[:], plosses[:])
            if sim:
                nc.sync.dma_start(lout[:], lin[:])
            else:
                nc.gpsimd.collective_compute(
                    "AllReduce", OP.add, replica_groups=[list(range(NCORES))],
                    ins=[lin[:].opt()], outs=[lout[:].opt()],
                )
            nc.sync.dma_start(losses_out.ap(), lout[:])

    nc.compile()
    return nc


_NC = None


def _get_nc():
    global _NC
    if _NC is None:
        _NC = build_program()
    return _NC


def kernel(x, conv_w, conv_b, bn_gamma, bn_beta, tr_w, tr_b, perm, _trace=False):
    x = np.asarray(x, np.float32)
    conv_w = np.asarray(conv_w, np.float32)
    conv_b = np.asarray(conv_b, np.float32)
    bn_gamma = np.asarray(bn_gamma, np.float32)
    bn_beta = np.asarray(bn_beta, np.float32)
    tr_w = np.ascontiguousarray(np.asarray(tr_w, np.float32))
    tr_b = np.ascontiguousarray(np.asarray(tr_b, np.float32))
    perm = np.asarray(perm)

    wc = np.ascontiguousarray(conv_w[:, 0, :].T)          # (10, 256)
    cb = np.ascontiguousarray(conv_b.reshape(C, 1))
    gm = np.ascontiguousarray(bn_gamma.reshape(C, 1))
    bt = np.ascontiguousarray(bn_beta.reshape(C, 1))
    perm32 = np.zeros((2048, 1), np.int32)
    perm32[:L, 0] = perm.astype(np.int32)

    in_maps = []
    for j in range(NCORES):
        xs = np.ascontiguousarray(
            np.stack([x[15 - 2 * j, 0, :], x[14 - 2 * j, 0, :]]))
        in_maps.append({
            "x_sh": xs, "wc": wc, "cb": cb, "gm": gm, "bt": bt,
            "trw": tr_w, "trb": tr_b, "perm32": perm32,
        })

    nc = _get_nc()
    res = run_bass_kernel_spmd(nc, in_maps, core_ids=list(range(NCORES)),
                               trace=_trace)

    z_full = np.empty((B, C, L), np.float32)
    for j in range(NCORES):
        zo = res.results[j]["z_out"]
        z_full[15 - 2 * j] = zo[0]
        z_full[14 - 2 * j] = zo[1]
    losses = np.ascontiguousarray(res.results[0]["losses"][:, 0])
    if _trace:
        kernel.last_exec_time_ns = res.exec_time_ns
    return z_full, losses
